# revision 26
# baseline (speedup 1.0000x reference)
"""Causal single-head attention (B=4, S=4096, D=1024, H=64) on 8 TRN2 cores.

Sharding: 8 cores = 4 batches x 2 query-fold roles. Queries and keys are
split in 512-row chunks, interleaved: role 0 owns chunks {0,2,4,6}, role 1
owns {1,3,5,7} of its batch. With this fold, same-fold (q,k) causal
geometry is role-independent, and cross-fold geometry differs only by an
all-ones/all-zeros mask on the fold-diagonal — so one SPMD program serves
all 8 cores, with the role differences carried entirely by tiny input
tensors (4 static diagonal masks + a [128,2] role-select).

mode="split3" (default): k/v raw inputs are fold-split per core (8 MiB
each); projected K^T and V (bf16) are exchanged within each batch pair via
one pairwise AllGather; attention on local keys overlaps the collective,
attention on peer keys runs after a role-select combine (peer = k0*n0 +
k1*n1 on DVE, n from input data).
mode="split": like split3 but all K/V read back from the blob (no overlap).
mode="replicate": full k/v per core, no collectives.

Projections run in f32r (tf32) on the PE; attention matmuls in bf16 with
fp32 PSUM accumulation. Softmax needs no running max (scores ~N(0,1));
denominator comes from an ones-column appended to V; division happens on
the host: output is oT [65, 2048] = [numerator^T; denominator].
"""

import numpy as np
import ml_dtypes

import concourse.bacc as bacc
import concourse.mybir as mybir
import concourse.tile as tile
from concourse.masks import make_identity
from concourse.bass_utils import run_bass_kernel_spmd

B, S, D, H = 4, 4096, 1024, 64
SBLK = 512          # q-tile width & projection S-block
NCH = D // 128      # 8 contraction chunks
QLOC = 2048         # q rows per core
NSLOT = QLOC // SBLK   # 4 q-tiles per core
NKB = S // 128      # 32 key blocks globally

F32 = mybir.dt.float32
F32R = mybir.dt.float32r
BF16 = mybir.dt.bfloat16

MODE = "split3"
RG_PAIRS = [[0, 1], [2, 3], [4, 5], [6, 7]]


def build_kernel(repeat: int = 1, phase: str = "full", mode: str = MODE, pair_exp: bool = False, rowpack: bool = False, defer0: bool = True, pp2: bool = True, vfirst: bool = False):
    kv_s = S if mode == "replicate" else QLOC
    nc = bacc.Bacc("TRN2", target_bir_lowering=False, debug=False, num_devices=8)

    qT = nc.dram_tensor("qT", [D, QLOC], F32R, kind="ExternalInput")
    kT = nc.dram_tensor("kT", [D, kv_s], F32R, kind="ExternalInput")
    vT = nc.dram_tensor("vT", [D, kv_s], F32R, kind="ExternalInput")
    wqT = nc.dram_tensor("wqT", [D, H], F32R, kind="ExternalInput")
    wkT = nc.dram_tensor("wkT", [D, H], F32R, kind="ExternalInput")
    wvT = nc.dram_tensor("wvT", [D, H], F32R, kind="ExternalInput")
    msk_n = 1 if mode == "split3" else 8
    masks = nc.dram_tensor("masks", [msk_n, 128, SBLK], BF16, kind="ExternalInput")
    rolesel = nc.dram_tensor("rolesel", [128, 2], F32, kind="ExternalInput")
    out = nc.dram_tensor("oT", [H + 1, QLOC], F32, kind="ExternalOutput")

    with tile.TileContext(nc) as tc:
        with (
            tc.tile_pool(name="const", bufs=1) as const_pool,
            tc.tile_pool(name="big", bufs=1) as big_pool,
            tc.tile_pool(name="strips", bufs=3) as strip_pool,
            tc.tile_pool(name="work", bufs=12) as work_pool,
            tc.tile_pool(name="pp", bufs=2 if pp2 else 1, space="PSUM") as pp,
            tc.tile_pool(name="pvt", bufs=1, space="PSUM") as pvt,
            tc.tile_pool(name="ps_sc", bufs=2 if pair_exp else (3 if defer0 else 2), space="PSUM") as ps_sc,
            tc.tile_pool(name="ps_o", bufs=2 if pair_exp else (3 if defer0 else 4), space="PSUM") as ps_o,
            tc.tile_pool(name="dram", bufs=1, space="DRAM") as dram_pool,
        ):
            # ---- constants ----
            wq_sb = const_pool.tile([128, NCH, H], F32R)
            wk_sb = const_pool.tile([128, NCH, H], F32R)
            wv_sb = const_pool.tile([128, NCH, H], F32R)
            nc.sync.dma_start(wq_sb[:], wqT.rearrange("(c p) h -> p c h", p=128))
            nc.sync.dma_start(wk_sb[:], wkT.rearrange("(c p) h -> p c h", p=128))
            nc.sync.dma_start(wv_sb[:], wvT.rearrange("(c p) h -> p c h", p=128))
            mask_sb = const_pool.tile([128, msk_n, SBLK], BF16)
            nc.sync.dma_start(mask_sb[:], masks.rearrange("m p q -> p m q"))
            rs = const_pool.tile([128, 2], F32)
            nc.sync.dma_start(rs[:], rolesel[:])
            ident = const_pool.tile([128, 128], F32)
            make_identity(nc, ident[:])

            # ---- persistent per-pass tensors ----
            qt_sb = big_pool.tile([128 if rowpack else H, QLOC], BF16)
            ot_sb = big_pool.tile([H + 1, QLOC], F32)
            if mode == "replicate":
                kt_sb = big_pool.tile([H, S], BF16)
                vaug = big_pool.tile([128, NKB, H + 1], BF16)
            elif mode == "split":
                kt_sb = big_pool.tile([H, S], BF16)
                vaug = big_pool.tile([128, NKB, H + 1], BF16)
                kt_loc = big_pool.tile([H, QLOC], BF16)
                vloc = big_pool.tile([128, QLOC // 128, H], BF16)
            else:  # split3
                ktp = 128 if rowpack else H
                kt_loc = big_pool.tile([ktp, QLOC], BF16)
                kt_peer = big_pool.tile([ktp, QLOC], BF16)
                vloc_aug = big_pool.tile([128, 16, H + 1], BF16)
                vpeer_aug = big_pool.tile([128, 16, H + 1], BF16)
                k_sc0 = big_pool.tile([H, QLOC], BF16)
                k_sc1 = big_pool.tile([H, QLOC], BF16)
                v_sc0 = big_pool.tile([128, 16, H], BF16)
                v_sc1 = big_pool.tile([128, 16, H], BF16)
                vpeer_diag = big_pool.tile([128, 16, H + 1], BF16)

            const_pt = const_pool.tile([128, SBLK], BF16)
            nc.vector.memset(const_pt[:], 0.001)

            for _rep in range(repeat):

                def load_strip(src_dram, s_off, tag="xstrip"):
                    strip = strip_pool.tile([128, NCH, SBLK], F32R, tag=tag)
                    nc.sync.dma_start(
                        strip[:],
                        src_dram[:, s_off : s_off + SBLK].rearrange(
                            "(c p) s -> p c s", p=128
                        ),
                    )
                    return strip

                def project_mm(w_sb, strip):
                    psum = pp.tile([H, SBLK], F32, tag="proj")
                    for c in range(NCH):
                        nc.tensor.matmul(
                            psum[:],
                            w_sb[:, c, :],
                            strip[:, c, :],
                            start=(c == 0),
                            stop=(c == NCH - 1),
                        )
                    return psum

                def project(dst_bf16, w_sb, src_dram, s_off, dst_off):
                    strip = load_strip(src_dram, s_off)
                    psum = project_mm(w_sb, strip)
                    nc.vector.tensor_copy(
                        dst_bf16[:, dst_off : dst_off + SBLK], psum[:]
                    )

                def project_v(dst_nat, src_dram, s_off, tile_off):
                    strip = load_strip(src_dram, s_off)
                    vpsum = project_mm(wv_sb, strip)
                    vt_stage = work_pool.tile([H, SBLK], F32, tag="vt_stage")
                    nc.vector.tensor_copy(vt_stage[:], vpsum[:])
                    for i in range(SBLK // 128):
                        if pp2:
                            tp = pp.tile([128, H], F32, tag="proj", name="tp")
                        else:
                            tp = pvt.tile([128, H], F32, tag="vtrans", name="tp")
                        nc.tensor.transpose(
                            tp[:], vt_stage[:, i * 128 : (i + 1) * 128], ident[:H, :H]
                        )
                        nc.vector.tensor_copy(dst_nat[:, tile_off + i, :H], tp[:])

                def attn_block(po, kt, kslice, va, vtile, s, maskop, first, last,
                               trim=0):
                    w = SBLK - trim
                    ps = ps_sc.tile([128, SBLK], F32, tag="scores")
                    if rowpack and kslice % 2 == 1:
                        lhsT = kt[64:128, kslice * 128 : (kslice + 1) * 128]
                        rhs = qt_sb[64:128, s * SBLK + trim : (s + 1) * SBLK]
                    else:
                        lhsT = kt[0:64, kslice * 128 : (kslice + 1) * 128]
                        rhs = qt_sb[0:64, s * SBLK + trim : (s + 1) * SBLK]
                    nc.tensor.matmul(
                        ps[:, :w], lhsT, rhs, start=True, stop=True,
                    )
                    if phase == "noexp":
                        pt = const_pt
                        nc.tensor.matmul(
                            po[:], va[:, vtile, :], pt[:], start=first, stop=last
                        )
                        return
                    pt = work_pool.tile([128, SBLK], BF16, tag="pt")
                    nc.scalar.activation(
                        pt[:, :w], ps[:, :w], mybir.ActivationFunctionType.Exp
                    )
                    if maskop == "rolesel":
                        nc.vector.tensor_scalar_mul(pt[:, :w], pt[:, :w], rs[:, 0:1])
                    elif maskop is not None:
                        nc.vector.tensor_mul(pt[:, :w], pt[:, :w], maskop)
                    nc.tensor.matmul(
                        po[:, trim:], va[:, vtile, :], pt[:, :w],
                        start=first, stop=last,
                    )

                def attn_pair(po, kt, kb2, va, s, maskkind, first, last):
                    # two consecutive 128-key blocks, one [128,1024] exp
                    ps2 = ps_sc.tile([128, 2, SBLK], F32, tag="scores")
                    for u in range(2):
                        nc.tensor.matmul(
                            ps2[:, u, :],
                            kt[:, (kb2 + u) * 128 : (kb2 + u + 1) * 128],
                            qt_sb[:, s * SBLK : (s + 1) * SBLK],
                            start=True,
                            stop=True,
                        )
                    pt2 = work_pool.tile([128, 2, SBLK], BF16, tag="pt")
                    nc.scalar.activation(
                        pt2[:], ps2[:], mybir.ActivationFunctionType.Exp
                    )
                    if maskkind == "rolesel":
                        nc.vector.tensor_scalar_mul(pt2[:], pt2[:], rs[:, 0:1])
                    elif maskkind == "diag":
                        i = kb2 % 4
                        nc.vector.tensor_mul(
                            pt2[:], pt2[:], mask_sb[:, i : i + 2, :]
                        )
                    for u in range(2):
                        nc.tensor.matmul(
                            po[:],
                            va[:, kb2 + u, :],
                            pt2[:, u, :],
                            start=(first and u == 0),
                            stop=(last and u == 1),
                        )

                if phase == "dma":
                    n_kv = kv_s // SBLK
                    for g in range(max(n_kv, NSLOT)):
                        srcs = ([kT, vT] if g < n_kv else []) + (
                            [qT] if g < NSLOT else []
                        )
                        for src in srcs:
                            strip = load_strip(src, g * SBLK)
                            nc.vector.tensor_copy(
                                ot_sb[:1, :8], strip[:1, 0, :8].bitcast(F32)
                            )
                    nc.vector.memset(ot_sb[:], 0.0)
                    nc.sync.dma_start(out[:], ot_sb[:])
                    continue

                if mode == "replicate":
                    nc.vector.memset(vaug[:, :, H], 1.0)
                    for g in range(S // SBLK):
                        off = g * SBLK
                        project(kt_sb, wk_sb, kT, off, off)
                        project_v(vaug, vT, off, off // 128)
                        if g < NSLOT:
                            project(qt_sb, wq_sb, qT, off, off)
                elif mode == "split":
                    nc.vector.memset(vaug[:, :, H], 1.0)
                    for g in range(NSLOT):
                        project(kt_loc, wk_sb, kT, g * SBLK, g * SBLK)
                    k_in_b = dram_pool.tile([H, QLOC], BF16, tag="k_in")
                    k_out_b = dram_pool.tile([2, H, QLOC], BF16, tag="k_out")
                    nc.gpsimd.dma_start(k_in_b[:], kt_loc[:H])
                    nc.gpsimd.collective_compute(
                        "AllGather", mybir.AluOpType.bypass,
                        replica_groups=RG_PAIRS,
                        ins=[k_in_b.opt()], outs=[k_out_b.opt()],
                    )
                    for g in range(NSLOT):
                        project_v(vloc, vT, g * SBLK, g * 4)
                    v_in_b = dram_pool.tile([128, QLOC // 128, H], BF16, tag="v_in")
                    v_out_b = dram_pool.tile(
                        [2, 128, QLOC // 128, H], BF16, tag="v_out"
                    )
                    nc.gpsimd.dma_start(v_in_b[:], vloc[:])
                    nc.gpsimd.collective_compute(
                        "AllGather", mybir.AluOpType.bypass,
                        replica_groups=RG_PAIRS,
                        ins=[v_in_b.opt()], outs=[v_out_b.opt()],
                    )
                    for g in range(NSLOT):
                        project(qt_sb, wq_sb, qT, g * SBLK, g * SBLK)
                    for rho in range(2):
                        for g in range(NSLOT):
                            nc.sync.dma_start(
                                kt_sb[:, SBLK * (2 * g + rho) :][:, :SBLK],
                                k_out_b[rho, :, g * SBLK : (g + 1) * SBLK],
                            )
                            nc.sync.dma_start(
                                vaug[:, 4 * (2 * g + rho) : 4 * (2 * g + rho) + 4, :H],
                                v_out_b[rho, :, 4 * g : 4 * g + 4, :],
                            )
                elif mode == "split3":
                    nc.vector.memset(vloc_aug[:, :, H], 1.0)
                    nc.vector.memset(vpeer_aug[:, :, H], 1.0)
                    # K projection first, AllGather(K) ASAP
                    for g in range(NSLOT):
                        project(kt_loc[:H], wk_sb, kT, g * SBLK, g * SBLK)
                    if rowpack:
                        nc.sync.dma_start(kt_loc[64:128, :], kt_loc[0:64, :])
                    k_in_b = dram_pool.tile([H, QLOC], BF16, tag="k_in")
                    k_out_b = dram_pool.tile([2, H, QLOC], BF16, tag="k_out")
                    nc.gpsimd.dma_start(k_in_b[:], kt_loc[:H])
                    nc.gpsimd.collective_compute(
                        "AllGather", mybir.AluOpType.bypass,
                        replica_groups=RG_PAIRS,
                        ins=[k_in_b.opt()], outs=[k_out_b.opt()],
                    )
                    # Q + V projections (vfirst: all V then AG_V then Q)
                    v_in_b = dram_pool.tile([128, 16, H], BF16, tag="v_in")
                    v_out_b = dram_pool.tile([2, 128, 16, H], BF16, tag="v_out")
                    if vfirst:
                        for g in range(NSLOT):
                            project_v(vloc_aug, vT, g * SBLK, g * 4)
                        nc.gpsimd.dma_start(v_in_b[:], vloc_aug[:, :, :H])
                        nc.gpsimd.collective_compute(
                            "AllGather", mybir.AluOpType.bypass,
                            replica_groups=RG_PAIRS,
                            ins=[v_in_b.opt()], outs=[v_out_b.opt()],
                        )
                        for g in range(NSLOT):
                            project(qt_sb[:H], wq_sb, qT, g * SBLK, g * SBLK)
                    else:
                        for g in range(NSLOT):
                            project(qt_sb[:H], wq_sb, qT, g * SBLK, g * SBLK)
                            project_v(vloc_aug, vT, g * SBLK, g * 4)
                        nc.gpsimd.dma_start(v_in_b[:], vloc_aug[:, :, :H])
                        nc.gpsimd.collective_compute(
                            "AllGather", mybir.AluOpType.bypass,
                            replica_groups=RG_PAIRS,
                            ins=[v_in_b.opt()], outs=[v_out_b.opt()],
                        )
                    if rowpack:
                        nc.sync.dma_start(qt_sb[64:128, :], qt_sb[0:64, :])
                    # same-fold attention (overlaps collectives)
                    nslots = NSLOT if phase in ("full", "noexp") else 0
                    pos = {}

                    def same_fold(s, po):
                        if pair_exp:
                            for g in range(s + 1):
                                for h2 in range(2):
                                    attn_pair(
                                        po, kt_loc, 4 * g + 2 * h2, vloc_aug, s,
                                        "diag" if g == s else None,
                                        first=(g == 0 and h2 == 0), last=False,
                                    )
                        else:
                            for g in range(s + 1):
                                for i in range(4):
                                    trim = 128 * i if g == s else 0
                                    maskop = (
                                        mask_sb[:, 0, : SBLK - trim]
                                        if g == s
                                        else None
                                    )
                                    attn_block(
                                        po, kt_loc, 4 * g + i, vloc_aug, 4 * g + i,
                                        s, maskop,
                                        first=(g == 0 and i == 0), last=False,
                                        trim=trim,
                                    )

                    def cross_fold(s, po):
                        if pair_exp:
                            for g in range(s + 1):
                                for h2 in range(2):
                                    attn_pair(
                                        po, kt_peer, 4 * g + 2 * h2, vpeer_aug, s,
                                        "rolesel" if g == s else None,
                                        first=False, last=(g == s and h2 == 1),
                                    )
                        else:
                            for g in range(s + 1):
                                va = vpeer_diag if g == s else vpeer_aug
                                for i in range(4):
                                    attn_block(
                                        po, kt_peer, 4 * g + i, va, 4 * g + i,
                                        s, None,
                                        first=False, last=(g == s and i == 3),
                                    )

                    def epilogue(s, po):
                        nc.vector.tensor_copy(
                            ot_sb[:, s * SBLK : (s + 1) * SBLK], po[:]
                        )

                    pre_slots = (2, 3) if pair_exp else ((1, 2, 3) if defer0 else (0, 1, 2, 3))
                    for s in pre_slots:
                        if s < nslots:
                            po = ps_o.tile([H + 1, SBLK], F32, tag="oT")
                            pos[s] = po
                            same_fold(s, po)
                    # unpack both ranks (static), role-select combine
                    nc.sync.dma_start(k_sc0[:], k_out_b[0])
                    nc.sync.dma_start(k_sc1[:], k_out_b[1])
                    ktmp = work_pool.tile([H, QLOC], BF16, tag="ktmp")
                    nc.vector.tensor_scalar_mul(kt_peer[:H], k_sc0[:], rs[:H, 0:1])
                    nc.vector.tensor_scalar_mul(ktmp[:], k_sc1[:], rs[:H, 1:2])
                    nc.vector.tensor_add(kt_peer[:H], kt_peer[:H], ktmp[:])
                    if rowpack:
                        nc.sync.dma_start(kt_peer[64:128, :], kt_peer[0:64, :])
                    nc.sync.dma_start(v_sc0[:], v_out_b[0])
                    nc.sync.dma_start(v_sc1[:], v_out_b[1])
                    vtmp = work_pool.tile([128, 16, H], BF16, tag="vtmp")
                    nc.vector.tensor_scalar_mul(
                        vpeer_aug[:, :, :H], v_sc0[:], rs[:, 0:1]
                    )
                    nc.vector.tensor_scalar_mul(vtmp[:], v_sc1[:], rs[:, 1:2])
                    nc.vector.tensor_add(
                        vpeer_aug[:, :, :H], vpeer_aug[:, :, :H], vtmp[:]
                    )
                    nc.vector.tensor_scalar_mul(
                        vpeer_diag[:], vpeer_aug[:], rs[:, 0:1]
                    )
                    # cross-fold attention + epilogues
                    cross_order = (2, 0, 1, 3) if pair_exp else ((1, 0, 2, 3) if defer0 else (0, 1, 2, 3))
                    for s in cross_order:
                        if s >= nslots:
                            continue
                        if s not in pos:
                            po = ps_o.tile([H + 1, SBLK], F32, tag="oT")
                            pos[s] = po
                            same_fold(s, po)
                        cross_fold(s, pos[s])
                        epilogue(s, pos[s])
                        nc.sync.dma_start(
                            out[:, s * SBLK : (s + 1) * SBLK],
                            ot_sb[:, s * SBLK : (s + 1) * SBLK],
                        )

                # ---- attention (non-split3 modes) ----
                if mode != "split3":
                    for s in range(NSLOT if phase in ("full", "noexp") else 0):
                        po = ps_o.tile([H + 1, SBLK], F32, tag="oT")
                        nblk = 8 * s + 8
                        for j in range(nblk):
                            m = j - 8 * s
                            maskop = mask_sb[:, m, :] if m >= 0 else None
                            attn_block(
                                po, kt_sb, j, vaug, j, s, maskop,
                                first=(j == 0), last=(j == nblk - 1),
                            )
                        nc.vector.tensor_copy(
                            ot_sb[:, s * SBLK : (s + 1) * SBLK], po[:]
                        )

                if phase not in ("full", "noexp"):
                    nc.vector.memset(ot_sb[:], 0.0)
                    nc.sync.dma_start(out[:], ot_sb[:])
                elif mode != "split3":
                    nc.sync.dma_start(out[:], ot_sb[:])

    nc.compile()
    return nc


def fold_rows(r):
    return np.concatenate(
        [np.arange(512 * (2 * s + r), 512 * (2 * s + r) + 512) for s in range(4)]
    )


def make_in_maps(q, k, v, Wq, Wk, Wv, mode: str = MODE):
    """Build the 8 per-core input maps from full inputs."""
    scale = 1.0 / np.sqrt(np.float32(H))
    wqT = np.ascontiguousarray((Wq * scale).T.astype(np.float32))
    wkT = np.ascontiguousarray(Wk.T.astype(np.float32))
    wvT = np.ascontiguousarray(Wv.T.astype(np.float32))

    kk = np.arange(128)[:, None]
    qq = np.arange(SBLK)[None, :]
    diag = [
        (qq >= kk + 128 * m).astype(ml_dtypes.bfloat16) for m in range(4)
    ]
    masks_by_role = []
    for r in range(2):
        ms = np.zeros((8, 128, SBLK), dtype=ml_dtypes.bfloat16)
        if mode == "split3":
            ms = ms[:1]
            ms[0] = diag[0]
        else:
            for m in range(8):
                if r == 0:
                    if m < 4:
                        ms[m] = diag[m]
                else:
                    ms[m] = (
                        np.ones((128, SBLK), dtype=ml_dtypes.bfloat16)
                        if m < 4
                        else diag[m - 4]
                    )
        masks_by_role.append(ms)

    in_maps = []
    for c in range(8):
        b, r = c // 2, c % 2
        qrows = fold_rows(r)
        kvrows = np.arange(S) if mode == "replicate" else qrows
        rsel = np.zeros((128, 2), dtype=np.float32)
        rsel[:, 0] = 1.0 if r == 1 else 0.0   # n0: peer is rank0 <=> I'm role1
        rsel[:, 1] = 1.0 if r == 0 else 0.0
        in_maps.append(
            {
                "qT": np.ascontiguousarray(q[b][qrows].T),
                "kT": np.ascontiguousarray(k[b][kvrows].T),
                "vT": np.ascontiguousarray(v[b][kvrows].T),
                "wqT": wqT,
                "wkT": wkT,
                "wvT": wvT,
                "masks": masks_by_role[r],
                "rolesel": rsel,
            }
        )
    return in_maps


def assemble_output(results):
    """results: list of 8 dicts with 'oT' [65, 2048] -> full [B, S, H]."""
    out = np.zeros((B, S, H), dtype=np.float32)
    for c in range(8):
        b, r = c // 2, c % 2
        oT = results[c]["oT"]
        for s in range(4):
            num = oT[:H, s * SBLK : (s + 1) * SBLK]
            den = oT[H, s * SBLK : (s + 1) * SBLK]
            g = 512 * (2 * s + r)
            out[b, g : g + 512, :] = (num / den[None, :]).T
    return out


_NC_CACHE = {}


def kernel(q, k, v, Wq, Wk, Wv):
    q = np.asarray(q, dtype=np.float32)
    k = np.asarray(k, dtype=np.float32)
    v = np.asarray(v, dtype=np.float32)
    Wq = np.asarray(Wq, dtype=np.float32)
    Wk = np.asarray(Wk, dtype=np.float32)
    Wv = np.asarray(Wv, dtype=np.float32)

    if "nc" not in _NC_CACHE:
        _NC_CACHE["nc"] = build_kernel()
    nc = _NC_CACHE["nc"]
    in_maps = make_in_maps(q, k, v, Wq, Wk, Wv)
    last_exc = None
    for attempt in range(3):
        try:
            res = run_bass_kernel_spmd(nc, in_maps, core_ids=list(range(8)))
            return assemble_output(res.results)
        except Exception as e:  # transient device/mesh issues: retry
            last_exc = e
            import time as _time

            _time.sleep(15 * (attempt + 1))
    raise last_exc


# revision 29
# speedup vs baseline: 817.9028x; 817.9028x over previous
"""Causal single-head attention (B=4, S=4096, D=1024, H=64) on 8 TRN2 cores.

Sharding: 8 cores = 4 batches x 2 query-fold roles. Queries and keys are
split in 512-row chunks, interleaved: role 0 owns chunks {0,2,4,6}, role 1
owns {1,3,5,7} of its batch. With this fold, same-fold (q,k) causal
geometry is role-independent, and cross-fold geometry differs only by an
all-ones/all-zeros mask on the fold-diagonal — so one SPMD program serves
all 8 cores, with the role differences carried entirely by tiny input
tensors (4 static diagonal masks + a [128,2] role-select).

mode="split3" (default): k/v raw inputs are fold-split per core (8 MiB
each); projected K^T and V (bf16) are exchanged within each batch pair via
one pairwise AllGather; attention on local keys overlaps the collective,
attention on peer keys runs after a role-select combine (peer = k0*n0 +
k1*n1 on DVE, n from input data).
mode="split": like split3 but all K/V read back from the blob (no overlap).
mode="replicate": full k/v per core, no collectives.

Projections run in f32r (tf32) on the PE; attention matmuls in bf16 with
fp32 PSUM accumulation. Softmax needs no running max (scores ~N(0,1));
denominator comes from an ones-column appended to V; division happens on
the host: output is oT [65, 2048] = [numerator^T; denominator].
"""

import numpy as np
import ml_dtypes

import concourse.bacc as bacc
import concourse.mybir as mybir
import concourse.tile as tile
from concourse.masks import make_identity
from concourse.bass_utils import run_bass_kernel_spmd

B, S, D, H = 4, 4096, 1024, 64
SBLK = 512          # q-tile width & projection S-block
NCH = D // 128      # 8 contraction chunks
QLOC = 2048         # q rows per core
NSLOT = QLOC // SBLK   # 4 q-tiles per core
NKB = S // 128      # 32 key blocks globally

F32 = mybir.dt.float32
F32R = mybir.dt.float32r
BF16 = mybir.dt.bfloat16

MODE = "split3"
RG_PAIRS = [[0, 1], [2, 3], [4, 5], [6, 7]]


def build_kernel(repeat: int = 1, phase: str = "full", mode: str = MODE, pair_exp: bool = False, rowpack: bool = False, defer0: bool = True, pp2: bool = True, vfirst: bool = False, chain2: bool = False):
    kv_s = S if mode == "replicate" else QLOC
    nc = bacc.Bacc("TRN2", target_bir_lowering=False, debug=False, num_devices=8)

    qT = nc.dram_tensor("qT", [D, QLOC], F32R, kind="ExternalInput")
    kT = nc.dram_tensor("kT", [D, kv_s], F32R, kind="ExternalInput")
    vT = nc.dram_tensor("vT", [D, kv_s], F32R, kind="ExternalInput")
    wqT = nc.dram_tensor("wqT", [D, H], F32R, kind="ExternalInput")
    wkT = nc.dram_tensor("wkT", [D, H], F32R, kind="ExternalInput")
    wvT = nc.dram_tensor("wvT", [D, H], F32R, kind="ExternalInput")
    msk_n = 1 if mode == "split3" else 8
    masks = nc.dram_tensor("masks", [msk_n, 128, SBLK], BF16, kind="ExternalInput")
    rolesel = nc.dram_tensor("rolesel", [128, 2], F32, kind="ExternalInput")
    out = nc.dram_tensor("oT", [H + 1, QLOC], F32, kind="ExternalOutput")

    with tile.TileContext(nc) as tc:
        with (
            tc.tile_pool(name="const", bufs=1) as const_pool,
            tc.tile_pool(name="big", bufs=1) as big_pool,
            tc.tile_pool(name="strips", bufs=4) as strip_pool,
            tc.tile_pool(name="work", bufs=2) as work_pool,
            tc.tile_pool(name="pp", bufs=2 if pp2 else 1, space="PSUM") as pp,
            tc.tile_pool(name="pvt", bufs=1, space="PSUM") as pvt,
            tc.tile_pool(name="ps_sc", bufs=2 if pair_exp else (3 if defer0 else 2), space="PSUM") as ps_sc,
            tc.tile_pool(name="ps_o", bufs=2 if pair_exp else (3 if defer0 else 4), space="PSUM") as ps_o,
            tc.tile_pool(name="dram", bufs=1, space="DRAM") as dram_pool,
        ):
            # ---- constants ----
            wq_sb = const_pool.tile([128, NCH, H], F32R)
            wk_sb = const_pool.tile([128, NCH, H], F32R)
            wv_sb = const_pool.tile([128, NCH, H], F32R)
            nc.sync.dma_start(wq_sb[:], wqT.rearrange("(c p) h -> p c h", p=128))
            nc.sync.dma_start(wk_sb[:], wkT.rearrange("(c p) h -> p c h", p=128))
            nc.sync.dma_start(wv_sb[:], wvT.rearrange("(c p) h -> p c h", p=128))
            mask_sb = const_pool.tile([128, msk_n, SBLK], BF16)
            nc.sync.dma_start(mask_sb[:], masks.rearrange("m p q -> p m q"))
            rs = const_pool.tile([128, 2], F32)
            nc.sync.dma_start(rs[:], rolesel[:])
            ident = const_pool.tile([128, 128], F32)
            make_identity(nc, ident[:])

            # ---- persistent per-pass tensors ----
            qt_sb = big_pool.tile([128 if rowpack else H, QLOC], BF16)
            ot_sb = big_pool.tile([H + 1, QLOC], F32)
            if mode == "replicate":
                kt_sb = big_pool.tile([H, S], BF16)
                vaug = big_pool.tile([128, NKB, H + 1], BF16)
            elif mode == "split":
                kt_sb = big_pool.tile([H, S], BF16)
                vaug = big_pool.tile([128, NKB, H + 1], BF16)
                kt_loc = big_pool.tile([H, QLOC], BF16)
                vloc = big_pool.tile([128, QLOC // 128, H], BF16)
            else:  # split3
                ktp = 128 if rowpack else H
                kt_loc = big_pool.tile([ktp, QLOC], BF16)
                kt_peer = big_pool.tile([ktp, QLOC], BF16)
                vloc_aug = big_pool.tile([128, 16, H + 1], BF16)
                vpeer_aug = big_pool.tile([128, 16, H + 1], BF16)
                k_sc0 = big_pool.tile([H, QLOC], BF16)
                k_sc1 = big_pool.tile([H, QLOC], BF16)
                v_sc0 = big_pool.tile([128, 16, H], BF16)
                v_sc1 = big_pool.tile([128, 16, H], BF16)
                vpeer_diag = big_pool.tile([128, 16, H + 1], BF16)

            const_pt = const_pool.tile([128, SBLK], BF16)
            nc.vector.memset(const_pt[:], 0.001)

            for _rep in range(repeat):

                def load_strip(src_dram, s_off, tag="xstrip"):
                    strip = strip_pool.tile([128, NCH, SBLK], F32R, tag=tag)
                    nc.sync.dma_start(
                        strip[:],
                        src_dram[:, s_off : s_off + SBLK].rearrange(
                            "(c p) s -> p c s", p=128
                        ),
                    )
                    return strip

                def project_mm(w_sb, strip):
                    psum = pp.tile([H, SBLK], F32, tag="proj")
                    for c in range(NCH):
                        nc.tensor.matmul(
                            psum[:],
                            w_sb[:, c, :],
                            strip[:, c, :],
                            start=(c == 0),
                            stop=(c == NCH - 1),
                        )
                    return psum

                def project(dst_bf16, w_sb, src_dram, s_off, dst_off):
                    strip = load_strip(src_dram, s_off)
                    psum = project_mm(w_sb, strip)
                    nc.vector.tensor_copy(
                        dst_bf16[:, dst_off : dst_off + SBLK], psum[:]
                    )

                def project_v(dst_nat, src_dram, s_off, tile_off):
                    strip = load_strip(src_dram, s_off)
                    vpsum = project_mm(wv_sb, strip)
                    vt_stage = work_pool.tile([H, SBLK], F32, tag="vt_stage")
                    nc.vector.tensor_copy(vt_stage[:], vpsum[:])
                    for i in range(SBLK // 128):
                        if pp2:
                            tp = pp.tile([128, H], F32, tag="proj", name="tp")
                        else:
                            tp = pvt.tile([128, H], F32, tag="vtrans", name="tp")
                        nc.tensor.transpose(
                            tp[:], vt_stage[:, i * 128 : (i + 1) * 128], ident[:H, :H]
                        )
                        nc.vector.tensor_copy(dst_nat[:, tile_off + i, :H], tp[:])

                def attn_block(po, kt, kslice, va, vtile, s, maskop, first, last,
                               trim=0):
                    w = SBLK - trim
                    ps = ps_sc.tile([128, SBLK], F32, tag="scores")
                    if rowpack and kslice % 2 == 1:
                        lhsT = kt[64:128, kslice * 128 : (kslice + 1) * 128]
                        rhs = qt_sb[64:128, s * SBLK + trim : (s + 1) * SBLK]
                    else:
                        lhsT = kt[0:64, kslice * 128 : (kslice + 1) * 128]
                        rhs = qt_sb[0:64, s * SBLK + trim : (s + 1) * SBLK]
                    nc.tensor.matmul(
                        ps[:, :w], lhsT, rhs, start=True, stop=True,
                    )
                    if phase == "noexp":
                        pt = const_pt
                        nc.tensor.matmul(
                            po[:], va[:, vtile, :], pt[:], start=first, stop=last
                        )
                        return
                    pt = work_pool.tile([128, SBLK], BF16, tag="pt", bufs=12)
                    nc.scalar.activation(
                        pt[:, :w], ps[:, :w], mybir.ActivationFunctionType.Exp
                    )
                    if maskop == "rolesel":
                        nc.vector.tensor_scalar_mul(pt[:, :w], pt[:, :w], rs[:, 0:1])
                    elif maskop is not None:
                        nc.vector.tensor_mul(pt[:, :w], pt[:, :w], maskop)
                    nc.tensor.matmul(
                        po[:, trim:], va[:, vtile, :], pt[:, :w],
                        start=first, stop=last,
                    )

                def attn_pair(po, kt, kb2, va, s, maskkind, first, last):
                    # two consecutive 128-key blocks, one [128,1024] exp
                    ps2 = ps_sc.tile([128, 2, SBLK], F32, tag="scores")
                    for u in range(2):
                        nc.tensor.matmul(
                            ps2[:, u, :],
                            kt[:, (kb2 + u) * 128 : (kb2 + u + 1) * 128],
                            qt_sb[:, s * SBLK : (s + 1) * SBLK],
                            start=True,
                            stop=True,
                        )
                    pt2 = work_pool.tile([128, 2, SBLK], BF16, tag="pt")
                    nc.scalar.activation(
                        pt2[:], ps2[:], mybir.ActivationFunctionType.Exp
                    )
                    if maskkind == "rolesel":
                        nc.vector.tensor_scalar_mul(pt2[:], pt2[:], rs[:, 0:1])
                    elif maskkind == "diag":
                        i = kb2 % 4
                        nc.vector.tensor_mul(
                            pt2[:], pt2[:], mask_sb[:, i : i + 2, :]
                        )
                    for u in range(2):
                        nc.tensor.matmul(
                            po[:],
                            va[:, kb2 + u, :],
                            pt2[:, u, :],
                            start=(first and u == 0),
                            stop=(last and u == 1),
                        )

                if phase == "dma":
                    n_kv = kv_s // SBLK
                    for g in range(max(n_kv, NSLOT)):
                        srcs = ([kT, vT] if g < n_kv else []) + (
                            [qT] if g < NSLOT else []
                        )
                        for src in srcs:
                            strip = load_strip(src, g * SBLK)
                            nc.vector.tensor_copy(
                                ot_sb[:1, :8], strip[:1, 0, :8].bitcast(F32)
                            )
                    nc.vector.memset(ot_sb[:], 0.0)
                    nc.sync.dma_start(out[:], ot_sb[:])
                    continue

                if mode == "replicate":
                    nc.vector.memset(vaug[:, :, H], 1.0)
                    for g in range(S // SBLK):
                        off = g * SBLK
                        project(kt_sb, wk_sb, kT, off, off)
                        project_v(vaug, vT, off, off // 128)
                        if g < NSLOT:
                            project(qt_sb, wq_sb, qT, off, off)
                elif mode == "split":
                    nc.vector.memset(vaug[:, :, H], 1.0)
                    for g in range(NSLOT):
                        project(kt_loc, wk_sb, kT, g * SBLK, g * SBLK)
                    k_in_b = dram_pool.tile([H, QLOC], BF16, tag="k_in")
                    k_out_b = dram_pool.tile([2, H, QLOC], BF16, tag="k_out")
                    nc.gpsimd.dma_start(k_in_b[:], kt_loc[:H])
                    nc.gpsimd.collective_compute(
                        "AllGather", mybir.AluOpType.bypass,
                        replica_groups=RG_PAIRS,
                        ins=[k_in_b.opt()], outs=[k_out_b.opt()],
                    )
                    for g in range(NSLOT):
                        project_v(vloc, vT, g * SBLK, g * 4)
                    v_in_b = dram_pool.tile([128, QLOC // 128, H], BF16, tag="v_in")
                    v_out_b = dram_pool.tile(
                        [2, 128, QLOC // 128, H], BF16, tag="v_out"
                    )
                    nc.gpsimd.dma_start(v_in_b[:], vloc[:])
                    nc.gpsimd.collective_compute(
                        "AllGather", mybir.AluOpType.bypass,
                        replica_groups=RG_PAIRS,
                        ins=[v_in_b.opt()], outs=[v_out_b.opt()],
                    )
                    for g in range(NSLOT):
                        project(qt_sb, wq_sb, qT, g * SBLK, g * SBLK)
                    for rho in range(2):
                        for g in range(NSLOT):
                            nc.sync.dma_start(
                                kt_sb[:, SBLK * (2 * g + rho) :][:, :SBLK],
                                k_out_b[rho, :, g * SBLK : (g + 1) * SBLK],
                            )
                            nc.sync.dma_start(
                                vaug[:, 4 * (2 * g + rho) : 4 * (2 * g + rho) + 4, :H],
                                v_out_b[rho, :, 4 * g : 4 * g + 4, :],
                            )
                elif mode == "split3":
                    nc.vector.memset(vloc_aug[:, :, H], 1.0)
                    nc.vector.memset(vpeer_aug[:, :, H], 1.0)
                    # K projection first, AllGather(K) ASAP
                    for g in range(NSLOT):
                        project(kt_loc[:H], wk_sb, kT, g * SBLK, g * SBLK)
                    if rowpack:
                        nc.sync.dma_start(kt_loc[64:128, :], kt_loc[0:64, :])
                    k_in_b = dram_pool.tile([H, QLOC], BF16, tag="k_in")
                    k_out_b = dram_pool.tile([2, H, QLOC], BF16, tag="k_out")
                    nc.gpsimd.dma_start(k_in_b[:], kt_loc[:H])
                    nc.gpsimd.collective_compute(
                        "AllGather", mybir.AluOpType.bypass,
                        replica_groups=RG_PAIRS,
                        ins=[k_in_b.opt()], outs=[k_out_b.opt()],
                    )
                    # Q + V projections (vfirst: all V then AG_V then Q)
                    v_in_b = dram_pool.tile([128, 16, H], BF16, tag="v_in")
                    v_out_b = dram_pool.tile([2, 128, 16, H], BF16, tag="v_out")
                    if vfirst:
                        for g in range(NSLOT):
                            project_v(vloc_aug, vT, g * SBLK, g * 4)
                        nc.gpsimd.dma_start(v_in_b[:], vloc_aug[:, :, :H])
                        nc.gpsimd.collective_compute(
                            "AllGather", mybir.AluOpType.bypass,
                            replica_groups=RG_PAIRS,
                            ins=[v_in_b.opt()], outs=[v_out_b.opt()],
                        )
                        for g in range(NSLOT):
                            project(qt_sb[:H], wq_sb, qT, g * SBLK, g * SBLK)
                    else:
                        qorder = (1, 2, 3, 0) if (defer0 and chain2) else range(NSLOT)
                        for gi, g in enumerate(qorder):
                            project(qt_sb[:H], wq_sb, qT, g * SBLK, g * SBLK)
                            project_v(vloc_aug, vT, gi * SBLK, gi * 4)
                        nc.gpsimd.dma_start(v_in_b[:], vloc_aug[:, :, :H])
                        nc.gpsimd.collective_compute(
                            "AllGather", mybir.AluOpType.bypass,
                            replica_groups=RG_PAIRS,
                            ins=[v_in_b.opt()], outs=[v_out_b.opt()],
                        )
                    if rowpack:
                        nc.sync.dma_start(qt_sb[64:128, :], qt_sb[0:64, :])
                    # same-fold attention (overlaps collectives)
                    nslots = NSLOT if phase in ("full", "noexp") else 0
                    pos = {}

                    def same_fold(s, po):
                        if pair_exp:
                            for g in range(s + 1):
                                for h2 in range(2):
                                    attn_pair(
                                        po, kt_loc, 4 * g + 2 * h2, vloc_aug, s,
                                        "diag" if g == s else None,
                                        first=(g == 0 and h2 == 0), last=False,
                                    )
                        else:
                            for g in range(s + 1):
                                for i in range(4):
                                    trim = 128 * i if g == s else 0
                                    maskop = (
                                        mask_sb[:, 0, : SBLK - trim]
                                        if g == s
                                        else None
                                    )
                                    attn_block(
                                        po, kt_loc, 4 * g + i, vloc_aug, 4 * g + i,
                                        s, maskop,
                                        first=(g == 0 and i == 0), last=False,
                                        trim=trim,
                                    )

                    def cross_fold(s, po):
                        if pair_exp:
                            for g in range(s + 1):
                                for h2 in range(2):
                                    attn_pair(
                                        po, kt_peer, 4 * g + 2 * h2, vpeer_aug, s,
                                        "rolesel" if g == s else None,
                                        first=False, last=(g == s and h2 == 1),
                                    )
                        else:
                            for g in range(s + 1):
                                va = vpeer_diag if g == s else vpeer_aug
                                for i in range(4):
                                    attn_block(
                                        po, kt_peer, 4 * g + i, va, 4 * g + i,
                                        s, None,
                                        first=False, last=(g == s and i == 3),
                                    )

                    def epilogue(s, po):
                        nc.vector.tensor_copy(
                            ot_sb[:, s * SBLK : (s + 1) * SBLK], po[:]
                        )

                    pre_slots = (2, 3) if pair_exp else ((1, 2, 3) if defer0 else (0, 1, 2, 3))
                    for s in pre_slots:
                        if s < nslots:
                            po = ps_o.tile([H + 1, SBLK], F32, tag="oT")
                            pos[s] = po
                            same_fold(s, po)
                    # unpack both ranks (static), role-select combine
                    nc.sync.dma_start(k_sc0[:], k_out_b[0])
                    nc.sync.dma_start(k_sc1[:], k_out_b[1])
                    ktmp = work_pool.tile([H, QLOC], BF16, tag="ktmp")
                    kchunks = range(NSLOT) if chain2 else [slice(None)]
                    for gc in range(NSLOT) if chain2 else [None]:
                        sl = (
                            slice(gc * SBLK, (gc + 1) * SBLK)
                            if chain2
                            else slice(None)
                        )
                        nc.vector.tensor_scalar_mul(
                            kt_peer[:H, sl], k_sc0[:, sl], rs[:H, 0:1]
                        )
                        nc.vector.tensor_scalar_mul(
                            ktmp[:, sl], k_sc1[:, sl], rs[:H, 1:2]
                        )
                        nc.vector.tensor_add(
                            kt_peer[:H, sl], kt_peer[:H, sl], ktmp[:, sl]
                        )
                    if rowpack:
                        nc.sync.dma_start(kt_peer[64:128, :], kt_peer[0:64, :])
                    nc.sync.dma_start(v_sc0[:], v_out_b[0])
                    nc.sync.dma_start(v_sc1[:], v_out_b[1])
                    vtmp = work_pool.tile([128, 16, H], BF16, tag="vtmp")
                    nc.vector.tensor_scalar_mul(
                        vpeer_aug[:, :, :H], v_sc0[:], rs[:, 0:1]
                    )
                    nc.vector.tensor_scalar_mul(vtmp[:], v_sc1[:], rs[:, 1:2])
                    nc.vector.tensor_add(
                        vpeer_aug[:, :, :H], vpeer_aug[:, :, :H], vtmp[:]
                    )
                    nc.vector.tensor_scalar_mul(
                        vpeer_diag[:], vpeer_aug[:], rs[:, 0:1]
                    )
                    # cross-fold attention + epilogues
                    cross_order = (2, 0, 1, 3) if pair_exp else ((1, 0, 2, 3) if defer0 else (0, 1, 2, 3))
                    for s in cross_order:
                        if s >= nslots:
                            continue
                        if s not in pos:
                            po = ps_o.tile([H + 1, SBLK], F32, tag="oT")
                            pos[s] = po
                            same_fold(s, po)
                        cross_fold(s, pos[s])
                        epilogue(s, pos[s])
                        nc.sync.dma_start(
                            out[:, s * SBLK : (s + 1) * SBLK],
                            ot_sb[:, s * SBLK : (s + 1) * SBLK],
                        )

                # ---- attention (non-split3 modes) ----
                if mode != "split3":
                    for s in range(NSLOT if phase in ("full", "noexp") else 0):
                        po = ps_o.tile([H + 1, SBLK], F32, tag="oT")
                        nblk = 8 * s + 8
                        for j in range(nblk):
                            m = j - 8 * s
                            maskop = mask_sb[:, m, :] if m >= 0 else None
                            attn_block(
                                po, kt_sb, j, vaug, j, s, maskop,
                                first=(j == 0), last=(j == nblk - 1),
                            )
                        nc.vector.tensor_copy(
                            ot_sb[:, s * SBLK : (s + 1) * SBLK], po[:]
                        )

                if phase not in ("full", "noexp"):
                    nc.vector.memset(ot_sb[:], 0.0)
                    nc.sync.dma_start(out[:], ot_sb[:])
                elif mode != "split3":
                    nc.sync.dma_start(out[:], ot_sb[:])

    nc.compile()
    return nc


def fold_rows(r):
    return np.concatenate(
        [np.arange(512 * (2 * s + r), 512 * (2 * s + r) + 512) for s in range(4)]
    )


def make_in_maps(q, k, v, Wq, Wk, Wv, mode: str = MODE):
    """Build the 8 per-core input maps from full inputs."""
    scale = 1.0 / np.sqrt(np.float32(H))
    wqT = np.ascontiguousarray((Wq * scale).T.astype(np.float32))
    wkT = np.ascontiguousarray(Wk.T.astype(np.float32))
    wvT = np.ascontiguousarray(Wv.T.astype(np.float32))

    kk = np.arange(128)[:, None]
    qq = np.arange(SBLK)[None, :]
    diag = [
        (qq >= kk + 128 * m).astype(ml_dtypes.bfloat16) for m in range(4)
    ]
    masks_by_role = []
    for r in range(2):
        ms = np.zeros((8, 128, SBLK), dtype=ml_dtypes.bfloat16)
        if mode == "split3":
            ms = ms[:1]
            ms[0] = diag[0]
        else:
            for m in range(8):
                if r == 0:
                    if m < 4:
                        ms[m] = diag[m]
                else:
                    ms[m] = (
                        np.ones((128, SBLK), dtype=ml_dtypes.bfloat16)
                        if m < 4
                        else diag[m - 4]
                    )
        masks_by_role.append(ms)

    in_maps = []
    for c in range(8):
        b, r = c // 2, c % 2
        qrows = fold_rows(r)
        kvrows = np.arange(S) if mode == "replicate" else qrows
        rsel = np.zeros((128, 2), dtype=np.float32)
        rsel[:, 0] = 1.0 if r == 1 else 0.0   # n0: peer is rank0 <=> I'm role1
        rsel[:, 1] = 1.0 if r == 0 else 0.0
        in_maps.append(
            {
                "qT": np.ascontiguousarray(q[b][qrows].T),
                "kT": np.ascontiguousarray(k[b][kvrows].T),
                "vT": np.ascontiguousarray(v[b][kvrows].T),
                "wqT": wqT,
                "wkT": wkT,
                "wvT": wvT,
                "masks": masks_by_role[r],
                "rolesel": rsel,
            }
        )
    return in_maps


def assemble_output(results):
    """results: list of 8 dicts with 'oT' [65, 2048] -> full [B, S, H]."""
    out = np.zeros((B, S, H), dtype=np.float32)
    for c in range(8):
        b, r = c // 2, c % 2
        oT = results[c]["oT"]
        for s in range(4):
            num = oT[:H, s * SBLK : (s + 1) * SBLK]
            den = oT[H, s * SBLK : (s + 1) * SBLK]
            g = 512 * (2 * s + r)
            out[b, g : g + 512, :] = (num / den[None, :]).T
    return out


_NC_CACHE = {}


def kernel(q, k, v, Wq, Wk, Wv):
    q = np.asarray(q, dtype=np.float32)
    k = np.asarray(k, dtype=np.float32)
    v = np.asarray(v, dtype=np.float32)
    Wq = np.asarray(Wq, dtype=np.float32)
    Wk = np.asarray(Wk, dtype=np.float32)
    Wv = np.asarray(Wv, dtype=np.float32)

    if "nc" not in _NC_CACHE:
        _NC_CACHE["nc"] = build_kernel()
    nc = _NC_CACHE["nc"]
    in_maps = make_in_maps(q, k, v, Wq, Wk, Wv)
    last_exc = None
    for attempt in range(3):
        try:
            res = run_bass_kernel_spmd(nc, in_maps, core_ids=list(range(8)))
            return assemble_output(res.results)
        except Exception as e:  # transient device/mesh issues: retry
            last_exc = e
            import time as _time

            _time.sleep(15 * (attempt + 1))
    raise last_exc
